# revision 3
# baseline (speedup 1.0000x reference)
"""Trainium2 Bass kernel for the top-k hinge loss (nn_Loss3).

Math (per row b of x [B, C]):
    shifted = x + 1 everywhere except at the label y[b] (stays x[b, y[b]])
    ret[b]  = sum(relu(top5(shifted) - s_y)),   s_y = x[b, y[b]]
    out     = mean(ret / k)

Device algorithm (exact, incl. fp ties):
    v8 = top-8 values of row (DVE Max8, multiset, sorted desc)
    s_y gathered via indirect DMA
    c  = #{v_i > s_y}
    top-5 of non-label row = first 5 of (v8 with index-c element dropped when c<5)
    ret = sum over kept i of relu(v_i - (s_y - 1))

Sharding: data-parallel over rows, 1024 rows per core on 8 cores; host
averages the 8192 per-row sums.
"""

import numpy as np

import concourse.bass as bass
import concourse.mybir as mybir
from concourse import bacc, tile
from concourse.bass_utils import run_bass_kernel_spmd

B, C = 8192, 50257
K = 5
N_CORES = 8
ROWS = B // N_CORES          # 1024 rows per core
P = 128                      # SBUF partitions
RT = ROWS // P               # 8 row-tiles per core

F32 = mybir.dt.float32
U32 = mybir.dt.uint32
I32 = mybir.dt.int32
Alu = mybir.AluOpType


def _chunk_sizes(cols, max_chunk=16384):
    n = -(-cols // max_chunk)
    base = cols // n
    rem = cols - base * n
    return [base + (1 if i < rem else 0) for i in range(n)]


def build_nc(rows=ROWS, cols=C, big_bufs=3):
    rt = rows // P
    chunks = _chunk_sizes(cols)
    nchunks = len(chunks)

    nc = bacc.Bacc(None, target_bir_lowering=False)
    x_in = nc.dram_tensor("x", [rows, cols], F32, kind="ExternalInput")
    yoff = nc.dram_tensor("yoff", [rt, P, 1], U32, kind="ExternalInput")
    ret_out = nc.dram_tensor("ret", [rt, P, 1], F32, kind="ExternalOutput")

    x_flat = x_in.rearrange("r c -> (r c)")[:, None]

    with tile.TileContext(nc) as tc:
        with (
            tc.tile_pool(name="const", bufs=1) as cpool,
            tc.tile_pool(name="big", bufs=big_bufs) as bpool,
            tc.tile_pool(name="small", bufs=2) as spool,
        ):
            iota_i = cpool.tile([P, 8], I32)
            nc.gpsimd.iota(iota_i[:], pattern=[[1, 8]], base=0, channel_multiplier=0)
            iota_f = cpool.tile([P, 8], F32)
            nc.vector.tensor_copy(iota_f[:], iota_i[:])
            base5 = cpool.tile([P, 8], F32)
            nc.vector.memset(base5[:, 0:5], 1.0)
            nc.vector.memset(base5[:, 5:8], 0.0)
            e5 = cpool.tile([P, 8], F32)
            nc.vector.memset(e5[:, 0:5], 0.0)
            nc.vector.memset(e5[:, 5:6], 1.0)
            nc.vector.memset(e5[:, 6:8], 0.0)

            for t in range(rt):
                rows_slice = slice(t * P, (t + 1) * P)
                cand = spool.tile([P, 8 * nchunks], F32, tag="cand")
                c0 = 0
                for ci, w in enumerate(chunks):
                    xt = bpool.tile([P, max(chunks)], F32, tag="xt")
                    nc.sync.dma_start(
                        out=xt[:, :w], in_=x_in[rows_slice, c0 : c0 + w]
                    )
                    nc.vector.max(out=cand[:, ci * 8 : (ci + 1) * 8], in_=xt[:, :w])
                    c0 += w
                v8 = spool.tile([P, 8], F32, tag="v8")
                nc.vector.max(out=v8[:], in_=cand[:])

                yo = spool.tile([P, 1], U32, tag="yo")
                nc.sync.dma_start(out=yo[:], in_=yoff[t])
                sy = spool.tile([P, 1], F32, tag="sy")
                nc.gpsimd.indirect_dma_start(
                    out=sy[:],
                    out_offset=None,
                    in_=x_flat[:],
                    in_offset=bass.IndirectOffsetOnAxis(ap=yo[:, :1], axis=0),
                )

                # a = s_y - 1 (threshold for the hinge)
                a_t = spool.tile([P, 1], F32, tag="a")
                nc.vector.tensor_scalar(
                    out=a_t[:], in0=sy[:], scalar1=1.0, scalar2=None, op0=Alu.subtract
                )
                # c = #{v_i > s_y}
                gt = spool.tile([P, 8], F32, tag="gt")
                nc.vector.tensor_scalar(
                    out=gt[:], in0=v8[:], scalar1=sy[:, :1], scalar2=None, op0=Alu.is_gt
                )
                c_t = spool.tile([P, 1], F32, tag="c")
                nc.vector.tensor_reduce(
                    out=c_t[:], in_=gt[:], axis=mybir.AxisListType.X, op=Alu.add
                )
                # h_i = relu(v_i - a)
                h = spool.tile([P, 8], F32, tag="h")
                nc.vector.tensor_scalar(
                    out=h[:],
                    in0=v8[:],
                    scalar1=a_t[:, :1],
                    scalar2=0.0,
                    op0=Alu.subtract,
                    op1=Alu.max,
                )
                # m = base5 + (c < 5) * (e5 - ec); ec_i = (i == c)
                g5 = spool.tile([P, 1], F32, tag="g5")
                nc.vector.tensor_scalar(
                    out=g5[:], in0=c_t[:], scalar1=5.0, scalar2=None, op0=Alu.is_lt
                )
                ec = spool.tile([P, 8], F32, tag="ec")
                nc.vector.tensor_scalar(
                    out=ec[:],
                    in0=iota_f[:],
                    scalar1=c_t[:, :1],
                    scalar2=None,
                    op0=Alu.is_equal,
                )
                e5c = spool.tile([P, 8], F32, tag="e5c")
                nc.vector.tensor_tensor(
                    out=e5c[:], in0=e5[:], in1=ec[:], op=Alu.subtract
                )
                w_t = spool.tile([P, 8], F32, tag="w")
                nc.vector.tensor_scalar(
                    out=w_t[:], in0=e5c[:], scalar1=g5[:, :1], scalar2=None, op0=Alu.mult
                )
                m_t = spool.tile([P, 8], F32, tag="m")
                nc.vector.tensor_tensor(out=m_t[:], in0=base5[:], in1=w_t[:], op=Alu.add)
                # ret = sum(m * h)
                mh = spool.tile([P, 8], F32, tag="mh")
                ret_t = spool.tile([P, 1], F32, tag="ret")
                nc.vector.tensor_tensor(out=mh[:], in0=m_t[:], in1=h[:], op=Alu.mult)
                nc.vector.tensor_reduce(
                    out=ret_t[:], in_=mh[:], axis=mybir.AxisListType.X, op=Alu.add
                )
                nc.sync.dma_start(out=ret_out[t], in_=ret_t[:])

    nc.compile()
    return nc


_NC = None


def _get_nc():
    global _NC
    if _NC is None:
        _NC = build_nc()
    return _NC


def make_in_maps(x, y):
    x = np.ascontiguousarray(np.asarray(x, dtype=np.float32))
    y = np.asarray(y).astype(np.int64)
    assert x.shape == (B, C), x.shape
    assert y.shape == (B,), y.shape
    in_maps = []
    local_r = np.arange(ROWS, dtype=np.int64)
    for core in range(N_CORES):
        r0 = core * ROWS
        y_loc = y[r0 : r0 + ROWS]
        off = local_r * C + y_loc
        assert off.max() < 2**32
        in_maps.append(
            {
                "x": x[r0 : r0 + ROWS],
                "yoff": off.astype(np.uint32).reshape(RT, P, 1),
            }
        )
    return in_maps


def finish(results, k):
    rets = np.concatenate(
        [np.asarray(r["ret"], dtype=np.float32).reshape(ROWS) for r in results]
    )
    return np.asarray(np.mean(rets.astype(np.float64)) / k, dtype=np.float32)


def kernel(x, y, k):
    k = int(k)
    assert k == K, k
    nc = _get_nc()
    res = run_bass_kernel_spmd(nc, make_in_maps(x, y), core_ids=list(range(N_CORES)))
    return finish(res.results, k)


# revision 5
# speedup vs baseline: 197.5908x; 197.5908x over previous
"""Trainium2 Bass kernel for the top-k hinge loss (nn_Loss3).

Math (per row b of x [B, C]):
    shifted = x + 1 everywhere except at the label y[b] (stays x[b, y[b]])
    ret[b]  = sum(relu(top5(shifted) - s_y)),   s_y = x[b, y[b]]
    out     = mean(ret / k)

Device algorithm (exact, incl. fp ties):
    v8 = top-8 values of row (DVE Max8, multiset, sorted desc)
    s_y gathered via indirect DMA
    c  = #{v_i > s_y}
    top-5 of non-label row = first 5 of (v8 with index-c element dropped when c<5)
    ret = sum over kept i of relu(v_i - (s_y - 1))

Sharding: data-parallel over rows, 1024 rows per core on 8 cores; host
averages the 8192 per-row sums.
"""

import numpy as np

import concourse.bass as bass
import concourse.mybir as mybir
from concourse import bacc, tile
from concourse.bass_utils import run_bass_kernel_spmd

B, C = 8192, 50257
K = 5
N_CORES = 8
ROWS = B // N_CORES          # 1024 rows per core
P = 128                      # SBUF partitions
RT = ROWS // P               # 8 row-tiles per core

F32 = mybir.dt.float32
U32 = mybir.dt.uint32
I32 = mybir.dt.int32
Alu = mybir.AluOpType


def _chunk_sizes(cols, max_chunk=16384):
    n = -(-cols // max_chunk)
    base = cols // n
    rem = cols - base * n
    return [base + (1 if i < rem else 0) for i in range(n)]


def build_nc(rows=ROWS, cols=C, big_bufs=3, repeats=1):
    rt = rows // P
    chunks = _chunk_sizes(cols)
    nchunks = len(chunks)

    nc = bacc.Bacc(None, target_bir_lowering=False)
    x_in = nc.dram_tensor("x", [rows, cols], F32, kind="ExternalInput")
    yoff = nc.dram_tensor("yoff", [rt, P, 1], U32, kind="ExternalInput")
    ret_out = nc.dram_tensor("ret", [rt, P, 1], F32, kind="ExternalOutput")

    x_flat = x_in.rearrange("r c -> (r c)")[:, None]

    with tile.TileContext(nc) as tc:
        with (
            tc.tile_pool(name="const", bufs=1) as cpool,
            tc.tile_pool(name="big", bufs=big_bufs) as bpool,
            tc.tile_pool(name="small", bufs=2) as spool,
        ):
            iota_i = cpool.tile([P, 8], I32)
            nc.gpsimd.iota(iota_i[:], pattern=[[1, 8]], base=0, channel_multiplier=0)
            iota_f = cpool.tile([P, 8], F32)
            nc.vector.tensor_copy(iota_f[:], iota_i[:])
            base5 = cpool.tile([P, 8], F32)
            nc.vector.memset(base5[:, 0:5], 1.0)
            nc.vector.memset(base5[:, 5:8], 0.0)
            e5 = cpool.tile([P, 8], F32)
            nc.vector.memset(e5[:, 0:5], 0.0)
            nc.vector.memset(e5[:, 5:6], 1.0)
            nc.vector.memset(e5[:, 6:8], 0.0)

            for t in range(rt * repeats):
                t = t % rt
                rows_slice = slice(t * P, (t + 1) * P)
                cand = spool.tile([P, 8 * nchunks], F32, tag="cand")
                c0 = 0
                for ci, w in enumerate(chunks):
                    xt = bpool.tile([P, max(chunks)], F32, tag="xt")
                    nc.sync.dma_start(
                        out=xt[:, :w], in_=x_in[rows_slice, c0 : c0 + w]
                    )
                    nc.vector.max(out=cand[:, ci * 8 : (ci + 1) * 8], in_=xt[:, :w])
                    c0 += w
                v8 = spool.tile([P, 8], F32, tag="v8")
                nc.vector.max(out=v8[:], in_=cand[:])

                yo = spool.tile([P, 1], U32, tag="yo")
                nc.sync.dma_start(out=yo[:], in_=yoff[t])
                sy = spool.tile([P, 1], F32, tag="sy")
                nc.gpsimd.indirect_dma_start(
                    out=sy[:],
                    out_offset=None,
                    in_=x_flat[:],
                    in_offset=bass.IndirectOffsetOnAxis(ap=yo[:, :1], axis=0),
                )

                # a = s_y - 1 (threshold for the hinge)
                a_t = spool.tile([P, 1], F32, tag="a")
                nc.vector.tensor_scalar(
                    out=a_t[:], in0=sy[:], scalar1=1.0, scalar2=None, op0=Alu.subtract
                )
                # c = #{v_i > s_y}
                gt = spool.tile([P, 8], F32, tag="gt")
                nc.vector.tensor_scalar(
                    out=gt[:], in0=v8[:], scalar1=sy[:, :1], scalar2=None, op0=Alu.is_gt
                )
                c_t = spool.tile([P, 1], F32, tag="c")
                nc.vector.tensor_reduce(
                    out=c_t[:], in_=gt[:], axis=mybir.AxisListType.X, op=Alu.add
                )
                # h_i = relu(v_i - a)
                h = spool.tile([P, 8], F32, tag="h")
                nc.vector.tensor_scalar(
                    out=h[:],
                    in0=v8[:],
                    scalar1=a_t[:, :1],
                    scalar2=0.0,
                    op0=Alu.subtract,
                    op1=Alu.max,
                )
                # m = base5 + (c < 5) * (e5 - ec); ec_i = (i == c)
                g5 = spool.tile([P, 1], F32, tag="g5")
                nc.vector.tensor_scalar(
                    out=g5[:], in0=c_t[:], scalar1=5.0, scalar2=None, op0=Alu.is_lt
                )
                ec = spool.tile([P, 8], F32, tag="ec")
                nc.vector.tensor_scalar(
                    out=ec[:],
                    in0=iota_f[:],
                    scalar1=c_t[:, :1],
                    scalar2=None,
                    op0=Alu.is_equal,
                )
                e5c = spool.tile([P, 8], F32, tag="e5c")
                nc.vector.tensor_tensor(
                    out=e5c[:], in0=e5[:], in1=ec[:], op=Alu.subtract
                )
                w_t = spool.tile([P, 8], F32, tag="w")
                nc.vector.tensor_scalar(
                    out=w_t[:], in0=e5c[:], scalar1=g5[:, :1], scalar2=None, op0=Alu.mult
                )
                m_t = spool.tile([P, 8], F32, tag="m")
                nc.vector.tensor_tensor(out=m_t[:], in0=base5[:], in1=w_t[:], op=Alu.add)
                # ret = sum(m * h)
                mh = spool.tile([P, 8], F32, tag="mh")
                ret_t = spool.tile([P, 1], F32, tag="ret")
                nc.vector.tensor_tensor(out=mh[:], in0=m_t[:], in1=h[:], op=Alu.mult)
                nc.vector.tensor_reduce(
                    out=ret_t[:], in_=mh[:], axis=mybir.AxisListType.X, op=Alu.add
                )
                nc.sync.dma_start(out=ret_out[t], in_=ret_t[:])

    nc.compile()
    return nc


_NC = None


def _get_nc():
    global _NC
    if _NC is None:
        _NC = build_nc()
    return _NC


def make_in_maps(x, y):
    x = np.ascontiguousarray(np.asarray(x, dtype=np.float32))
    y = np.asarray(y).astype(np.int64)
    assert x.shape == (B, C), x.shape
    assert y.shape == (B,), y.shape
    in_maps = []
    local_r = np.arange(ROWS, dtype=np.int64)
    for core in range(N_CORES):
        r0 = core * ROWS
        y_loc = y[r0 : r0 + ROWS]
        off = local_r * C + y_loc
        assert off.max() < 2**32
        in_maps.append(
            {
                "x": x[r0 : r0 + ROWS],
                "yoff": off.astype(np.uint32).reshape(RT, P, 1),
            }
        )
    return in_maps


def finish(results, k):
    rets = np.concatenate(
        [np.asarray(r["ret"], dtype=np.float32).reshape(ROWS) for r in results]
    )
    return np.asarray(np.mean(rets.astype(np.float64)) / k, dtype=np.float32)


def kernel(x, y, k):
    k = int(k)
    assert k == K, k
    nc = _get_nc()
    res = run_bass_kernel_spmd(nc, make_in_maps(x, y), core_ids=list(range(N_CORES)))
    return finish(res.results, k)
